# revision 19
# baseline (speedup 1.0000x reference)
"""Trainium2 Bass kernel for BiologicalMultiHeadAttention.

Sharding (8 cores): core c -> (batch b = c//2, head-group g = c%2).
Each core: q/k/v projections, dense softmax attention over its 8 heads,
neuromodulation gate, partial out-projection over its 512 channels.
Host sums the two partial projections per batch and adds bo.

Design: the softmax exp stream on ScalarE (~283us for 33.5M exps) is the
critical path; PE work is scheduled to hide under it.
  - scores: bf16, both heads of a pair packed concurrently in the PE
    array (row-group tiling at partitions 0/64, K=64 each) -> ~2x.
  - all data bf16 (fp8 fails the 2e-2 budget: each quantized tensor in
    the multiplicative path contributes its full ~3.6% elementwise RMS).
  - loop: head-pair outer, 512-query blocks; per key-chunk jc:
    2 packed score MMs -> one exp (N=1024, psum ping-pong) -> attn*v.
    Ones-column in v gives the denominator row for free (M=65).
  - projections / gate / normalize / out-projection drip through an
    ordered pending queue, one item per jc step.
"""

import numpy as np
import ml_dtypes
from collections import deque

import concourse.bass as bass
import concourse.tile as tile
from concourse import bacc, mybir
from concourse.bass_utils import run_bass_kernel_spmd

F32 = mybir.dt.float32
F32R = mybir.dt.float32r
BF16 = mybir.dt.bfloat16
AF = mybir.ActivationFunctionType
ALU = mybir.AluOpType

P = 128


def build_nc(S=2048, E=1024, HL=8, D=64, num_devices=8):
    CH = HL * D        # 512 channels per core
    NE = E // P        # 8 input-channel chunks
    NC = CH // P       # 4 output chunks (= head pairs)
    NS = S // P        # 16 key chunks
    HM = E // 4        # 256 mlp hidden
    NH = HM // P       # 2
    QB = 512           # query block
    NQB = S // QB      # 4

    nc = bacc.Bacc("TRN2", target_bir_lowering=False, debug=False,
                   num_devices=num_devices)

    xT_d = nc.dram_tensor("xT", [E, S], BF16, kind="ExternalInput").ap()
    wqT_d = nc.dram_tensor("wqT", [E, CH], BF16, kind="ExternalInput").ap()
    wkT_d = nc.dram_tensor("wkT", [E, CH], BF16, kind="ExternalInput").ap()
    wvT_d = nc.dram_tensor("wvT", [E, CH], BF16, kind="ExternalInput").ap()
    wm1T_d = nc.dram_tensor("wm1T", [E, HM], BF16, kind="ExternalInput").ap()
    wm2T_d = nc.dram_tensor("wm2T", [HM, CH], BF16, kind="ExternalInput").ap()
    wo_d = nc.dram_tensor("wo", [CH, E], BF16, kind="ExternalInput").ap()
    bq_d = nc.dram_tensor("bq", [CH], F32, kind="ExternalInput").ap()
    bk_d = nc.dram_tensor("bk", [CH], F32, kind="ExternalInput").ap()
    bvr_d = nc.dram_tensor("bvr", [P, CH], F32, kind="ExternalInput").ap()
    bm1_d = nc.dram_tensor("bm1", [HM], F32, kind="ExternalInput").ap()
    bm2_d = nc.dram_tensor("bm2", [CH], F32, kind="ExternalInput").ap()
    # scal cols: dop, ser, nor, ace, attn_scale, attn_bias, 0, 0
    scal_d = nc.dram_tensor("scal", [P, 8], F32, kind="ExternalInput").ap()
    sel_d = nc.dram_tensor("sel", [2, P], F32, kind="ExternalInput").ap()
    out_d = nc.dram_tensor("out", [S, E], BF16, kind="ExternalOutput").ap()

    with tile.TileContext(nc) as tc:
        with (
            tc.tile_pool(name="const", bufs=1) as const,
            tc.tile_pool(name="exp", bufs=4) as exp_pool,
            tc.tile_pool(name="evp", bufs=3) as evp,
            tc.tile_pool(name="denp", bufs=4) as denp,
            tc.tile_pool(name="rdp", bufs=2) as rdp,
            tc.tile_pool(name="t1p", bufs=2) as t1p,
            tc.tile_pool(name="osp", bufs=3) as osp,
            tc.tile_pool(name="scp", bufs=2, space="PSUM") as scp,
            tc.tile_pool(name="accp", bufs=2, space="PSUM") as accp,
            tc.tile_pool(name="ps", bufs=2, space="PSUM") as ps,
        ):
            # ---------------- loads ----------------
            # preload the exp table set on ScalarE while DMAs run
            warm_a = const.tile([1, 8], F32, tag="warm_a")
            warm_b = const.tile([1, 8], F32, tag="warm_b")
            nc.vector.memset(warm_a[:], 0.0)
            nc.scalar.activation(warm_b[:], warm_a[:], AF.Exp, scale=1.0)

            def load_b(dram, chunks, name):
                t = const.tile([P, chunks], F32, tag=name)
                nc.scalar.dma_start(t[:], dram.rearrange("(c p) -> p c", p=P))
                return t

            def load_w(dram, chunks, width, name, eng):
                t = const.tile([P, chunks, width], BF16, tag=name)
                r = dram.rearrange("(o p) f -> o p f", p=P)
                for o in range(chunks):
                    eng.dma_start(t[:, o, :], r[o])
                return t

            # Startup is DMA-bandwidth-bound: strict criticality order on
            # the two HWDGE queues.  The first attention unit needs all of
            # x but only the m=0 column chunks of wk/wq; v follows; the
            # rest of the weights trail.
            scal = const.tile([P, 8], F32, tag="scal")
            nc.scalar.dma_start(scal[:], scal_d)
            bq = load_b(bq_d, NC, "bq")
            bk = load_b(bk_d, NC, "bk")

            xT = const.tile([P, NE, S], BF16, tag="xT")
            wkT = const.tile([P, NE, CH], BF16, tag="wkT")
            wqT = const.tile([P, NE, CH], BF16, tag="wqT")
            wvT = const.tile([P, NE, CH], BF16, tag="wvT")
            x_r = xT_d.rearrange("(o p) f -> o p f", p=P)
            for o in range(NE):
                (nc.sync if o % 2 == 0 else nc.scalar).dma_start(
                    xT[:, o, :], x_r[o])

            def load_wcols(t, dram, m0, m1, eng):
                eng.dma_start(
                    t[:, :, m0 * P:m1 * P],
                    dram[:, m0 * P:m1 * P].rearrange("(o p) f -> p o f", p=P))

            load_wcols(wkT, wkT_d, 0, 1, nc.scalar)
            load_wcols(wqT, wqT_d, 0, 1, nc.sync)
            wv_r = wvT_d.rearrange("(o p) f -> o p f", p=P)
            for o in range(NE):
                (nc.sync if o % 2 == 0 else nc.scalar).dma_start(
                    wvT[:, o, :], wv_r[o])
            bv_bc = const.tile([P, CH], F32, tag="bv_bc")
            nc.scalar.dma_start(bv_bc[:], bvr_d)
            load_wcols(wkT, wkT_d, 1, NC, nc.scalar)
            load_wcols(wqT, wqT_d, 1, NC, nc.sync)
            sel_raw = const.tile([2, P], F32, tag="sel_raw")
            nc.scalar.dma_start(sel_raw[:], sel_d)
            bm1 = load_b(bm1_d, NH, "bm1")
            bm2 = load_b(bm2_d, NC, "bm2")
            wm1T = load_w(wm1T_d, NE, HM, "wm1T", nc.scalar)
            wm2T = load_w(wm2T_d, NH, CH, "wm2T", nc.scalar)
            wo = load_w(wo_d, NC, E, "wo", nc.sync)

            # ---------------- scalar-derived constants ----------------
            # nm = (dop+ser+nor+ace)/4
            nm = const.tile([P, 2], F32, tag="nm")
            nc.vector.tensor_tensor(nm[:, 0:1], scal[:, 0:1], scal[:, 1:2], ALU.add)
            nc.vector.tensor_tensor(nm[:, 1:2], scal[:, 2:3], scal[:, 3:4], ALU.add)
            nc.vector.tensor_tensor(nm[:, 0:1], nm[:, 0:1], nm[:, 1:2], ALU.add)
            nc.vector.tensor_scalar_mul(nm[:, 0:1], nm[:, 0:1], 0.25)
            nm_g = nm[:, 0:1]

            # gate = psum*nm + (1 + nm*bm2)
            c1 = const.tile([P, NC], F32, tag="c1")
            nc.vector.tensor_tensor(c1[:], bm2[:], nm_g.to_broadcast([P, NC]), ALU.mult)
            nc.vector.tensor_scalar_add(c1[:], c1[:], 1.0)

            # bc = sel.T @ rd(=1/den rows) scaled by attn_scale
            sel_s = const.tile([2, P], F32R, tag="sel_s")
            nc.vector.tensor_scalar(sel_s[:], sel_raw[:], scal[0:2, 4:5],
                                    None, ALU.mult)
            ab1 = const.tile([P, 1], F32, tag="ab1")
            nc.vector.tensor_copy(ab1[:], scal[:, 5:6])

            # ---------------- persistent activations ----------------
            qT = const.tile([P, NC, S], BF16, tag="qT")
            kT = const.tile([P, NC, S], BF16, tag="kT")
            v_aug = const.tile([P, NS, HL, D + 1], BF16, tag="v_aug")
            h1T = const.tile([P, NH, S], BF16, tag="h1T")
            gateT = const.tile([P, NC, S], BF16, tag="gateT")
            attn_raw = const.tile([P, NC, S], BF16, tag="attn_raw")
            rstage = const.tile([P, HL, NQB, QB // P], F32R, tag="rstage")

            nc.vector.memset(v_aug[:, :, :, D:D + 1], 1.0)

            # ---------------- pending work queue ----------------
            pending = deque()   # (label, fn)
            emitted = set()

            def push(label, fn):
                pending.append((label, fn))

            def drip(n=1):
                for _ in range(n):
                    if pending:
                        label, fn = pending.popleft()
                        fn()
                        emitted.add(label)

            def ensure(label):
                while pending and label not in emitted:
                    lb, fn = pending.popleft()
                    fn()
                    emitted.add(lb)

            def drain():
                while pending:
                    lb, fn = pending.popleft()
                    fn()
                    emitted.add(lb)

            # ---------------- emitters ----------------
            # proj groups are split in two halves (4 k-chunks each) so a
            # drip item is ~0.9us of PE work.
            def kq_half(wT, m, g, half, cell, name):
                cols = slice(g * 512, (g + 1) * 512)
                if half == 0:
                    cell["pt"] = ps.tile([P, 512], F32, tag="ps",
                                         name=f"pj_{name}_{m}_{g}")
                pt = cell["pt"]
                for k in range(half * 4, half * 4 + 4):
                    nc.tensor.matmul(
                        pt[:], wT[:, k, m * P:(m + 1) * P],
                        xT[:, k, cols],
                        start=(k == 0), stop=(k == NE - 1))

            def kq_evict(dest, bias, m, g, cell):
                cols = slice(g * 512, (g + 1) * 512)
                nc.vector.tensor_scalar(
                    dest[:, m, cols], cell.pop("pt")[:], bias[:, m:m + 1],
                    None, ALU.add)

            def emit_kq_group(wT, dest, bias, m, g, name):
                cell = {}
                kq_half(wT, m, g, 0, cell, name)
                kq_half(wT, m, g, 1, cell, name)
                kq_evict(dest, bias, m, g, cell)

            def push_kq_group(wT, dest, bias, m, g, name):
                cell = {}
                push(f"{name}{m}g{g}a",
                     lambda: kq_half(wT, m, g, 0, cell, name))
                push(f"{name}{m}g{g}",
                     lambda: (kq_half(wT, m, g, 1, cell, name),
                              kq_evict(dest, bias, m, g, cell)))

            def emit_v_chunk(c):
                # v natural layout [seq, ch] + bias
                pt = ps.tile([P, 512], F32, tag="ps", name=f"v_{c}")
                for k in range(NE):
                    nc.tensor.matmul(
                        pt[:, 0:CH], xT[:, k, c * P:(c + 1) * P],
                        wvT[:, k, :],
                        start=(k == 0), stop=(k == NE - 1))
                nc.vector.tensor_tensor(
                    v_aug[:, c, :, 0:D],
                    pt[:, 0:CH].rearrange("p (h d) -> p h d", h=HL),
                    bv_bc.rearrange("p (h d) -> p h d", h=HL),
                    ALU.add)

            def h1_group(m, g, half, cell):
                cols = slice(g * 512, (g + 1) * 512)
                if half == 0:
                    cell["pt"] = ps.tile([P, 512], F32, tag="ps",
                                         name=f"h1_{m}_{g}")
                pt = cell["pt"]
                for k in range(half * 4, half * 4 + 4):
                    nc.tensor.matmul(
                        pt[:], wm1T[:, k, m * P:(m + 1) * P],
                        xT[:, k, cols],
                        start=(k == 0), stop=(k == NE - 1))
                if half == 1:
                    nc.vector.tensor_scalar(
                        h1T[:, m, cols], cell.pop("pt")[:],
                        bm1[:, m:m + 1], 0.0, ALU.add, ALU.max)

            def push_h1(g):
                for m in range(NH):
                    cell = {}
                    push(f"h1m{m}g{g}a",
                         lambda m=m, g=g, cell=cell: h1_group(m, g, 0, cell))
                    push(f"h1m{m}g{g}",
                         lambda m=m, g=g, cell=cell: h1_group(m, g, 1, cell))

            def emit_gate_group(m, g):
                cols = slice(g * 512, (g + 1) * 512)
                pt = ps.tile([P, 512], F32, tag="ps", name=f"g_{m}_{g}")
                for k in range(NH):
                    nc.tensor.matmul(
                        pt[:], wm2T[:, k, m * P:(m + 1) * P],
                        h1T[:, k, cols],
                        start=(k == 0), stop=(k == NH - 1))
                nc.vector.tensor_scalar(
                    gateT[:, m, cols], pt[:], nm_g, c1[:, m:m + 1],
                    ALU.mult, ALU.add)

            def push_gate(m, g):
                push(f"gm{m}g{g}", lambda m=m, g=g: emit_gate_group(m, g))

            def emit_tail1(pr, qb):
                sl2 = slice(2 * pr, 2 * pr + 2)
                with nc.allow_low_precision(reason="f32r==f32 bits; PE reads fp22"):
                    nc.vector.reciprocal(rstage[:, sl2, qb, :],
                                         rstage[:, sl2, qb, :])
                rd = rdp.tile([2, QB], F32R, tag="rd", name=f"rd_{pr}_{qb}")
                for hp in range(2):
                    nc.sync.dma_start(rd[hp:hp + 1, :],
                                      rstage[:, 2 * pr + hp, qb, :])
                return rd

            def emit_tail2(pr, qb, rd):
                qsl = slice(qb * QB, (qb + 1) * QB)
                bc = ps.tile([P, QB], F32, tag="ps", name=f"bc_{pr}_{qb}")
                nc.tensor.matmul(bc[:], sel_s[:], rd[:], start=True, stop=True)
                t1 = t1p.tile([P, QB], BF16, tag="t1", name=f"t1_{pr}_{qb}")
                nc.vector.tensor_tensor(t1[:], attn_raw[:, pr, qsl], bc[:],
                                        ALU.mult)
                nc.vector.tensor_scalar_add(t1[:], t1[:], ab1[:])
                nc.vector.tensor_tensor(attn_raw[:, pr, qsl], t1[:],
                                        gateT[:, pr, qsl], ALU.mult)

            def push_tail(pr, qb):
                cell = {}
                def i1(pr=pr, qb=qb):
                    cell["rd"] = emit_tail1(pr, qb)
                def i2(pr=pr, qb=qb):
                    emit_tail2(pr, qb, cell.pop("rd"))
                push(f"tl1_{pr}_{qb}", i1)
                push(f"tl2_{pr}_{qb}", i2)

            def push_outproj(qb, ks=None, accum=False, sfx=""):
                # ks: which attn_raw chunks to contract (default all);
                # accum: DMA-accumulate into out (same sync queue -> ordered)
                ks = list(range(NC)) if ks is None else ks
                for t in range(qb * NQB, (qb + 1) * NQB):
                    for n in range(E // 512):
                        def item(t=t, n=n):
                            pt = ps.tile([P, 512], F32, tag="ps",
                                         name=f"op{sfx}_{t}_{n}")
                            for i, k in enumerate(ks):
                                nc.tensor.matmul(
                                    pt[:],
                                    attn_raw[:, k, t * P:(t + 1) * P],
                                    wo[:, k, n * 512:(n + 1) * 512],
                                    start=(i == 0), stop=(i == len(ks) - 1))
                            ot = osp.tile([P, 512], BF16, tag="os",
                                          name=f"os{sfx}_{t}_{n}")
                            nc.vector.tensor_copy(ot[:], pt[:])
                            nc.sync.dma_start(
                                out_d[t * P:(t + 1) * P,
                                      n * 512:(n + 1) * 512], ot[:])
                        push(f"op{sfx}_{t}_{n}", item)

            # ---------------- attention ----------------
            def evict_unit(pr, qb, acc):
                qsl = slice(qb * QB, (qb + 1) * QB)
                nc.vector.tensor_copy(attn_raw[0:D, pr, qsl], acc[0][0:D, :])
                tmpv = evp.tile([D, QB], BF16, tag="ev", name=f"ev_{pr}_{qb}")
                nc.vector.tensor_copy(tmpv[:], acc[1][0:D, :])
                nc.sync.dma_start(attn_raw[D:P, pr, qsl], tmpv[:])
                for h in range(2):
                    den = denp.tile([1, QB], F32R, tag="den",
                                    name=f"dn_{pr}_{qb}_{h}")
                    nc.vector.tensor_copy(den[:], acc[h][D:D + 1, :])
                    nc.sync.dma_start(rstage[:, 2 * pr + h, qb, :], den[:])

            def attn_unit(pr, qb, inline=None):
                qsl = slice(qb * QB, (qb + 1) * QB)
                acc = [accp.tile([D + 1, QB], F32, tag="acc",
                                 name=f"acc_{pr}_{qb}_{h}") for h in range(2)]

                def attnv(jc, ext):
                    for h in range(2):
                        nc.tensor.matmul(
                            acc[h][:],
                            v_aug[:, jc, 2 * pr + h, 0:D + 1],
                            ext[:, jc % 2, h, :],
                            start=(jc == 0), stop=(jc == NS - 1))

                ex = prev_ex = None
                for jc in range(NS):
                    sc = scp.tile([P, 2, QB], F32, tag="sc",
                                  name=f"sc_{pr}_{qb}_{jc}")
                    for h in range(2):
                        hb = h * D
                        nc.tensor.matmul(
                            sc[:, h, :],
                            kT[hb:hb + D, pr, jc * P:(jc + 1) * P],
                            qT[hb:hb + D, pr, qsl],
                            start=True, stop=True)
                    if jc % 2 == 0:
                        prev_ex = ex
                        ex = exp_pool.tile([P, 2, 2, QB], BF16, tag="ex",
                                           name=f"ex_{pr}_{qb}_{jc}")
                    nc.scalar.activation(ex[:, jc % 2], sc[:], AF.Exp,
                                         scale=0.125)
                    # attn*v lags one step so PE never waits on the current
                    # exp -- the wait window runs filler work instead
                    if jc > 0:
                        attnv(jc - 1, ex if jc % 2 == 1 else prev_ex)
                    if inline is not None:
                        inline(jc)
                    else:
                        drip(1)
                attnv(NS - 1, ex)
                evict_unit(pr, qb, acc)

            # ---------------- schedule ----------------
            # pre-phase: minimum to start (pr0, qb0)
            emit_kq_group(wkT, kT, bk, 0, 0, "k")
            emit_kq_group(wqT, qT, bq, 0, 0, "q")

            def pr0qb0_inline(jc):
                # v chunks trail the scores by design: chunk c is needed by
                # the lagged attn*v at emission step c+1
                if jc == 0:
                    emit_v_chunk(0)
                    emit_v_chunk(1)
                if jc <= 13:
                    emit_v_chunk(jc + 2)
                if jc == 1:
                    emit_kq_group(wkT, kT, bk, 0, 1, "k")
                elif jc == 5:
                    emit_kq_group(wkT, kT, bk, 0, 2, "k")
                elif jc == 9:
                    emit_kq_group(wkT, kT, bk, 0, 3, "k")
                elif jc >= 14:
                    drip(1)

            def push_k(m, gs):
                for g in gs:
                    push_kq_group(wkT, kT, bk, m, g, "k")

            def push_q(pr, qb):
                push_kq_group(wqT, qT, bq, pr, qb, f"q{pr}@")

            UNITS = [(pr, qb) for pr in range(NC) for qb in range(NQB)]
            for i, (pr, qb) in enumerate(UNITS):
                # queue the NEXT unit's q projection so it drips during
                # this unit (its label is ensured at that unit's start)
                if i + 1 < len(UNITS):
                    push_q(*UNITS[i + 1])
                if pr == 0 and qb == 0:
                    attn_unit(0, 0, inline=pr0qb0_inline)
                else:
                    if qb == 0:
                        ensure(f"k{pr}g{NQB - 1}")
                    ensure(f"q{pr}@{pr}g{qb}")
                    attn_unit(pr, qb)

                # pushes after unit (pr, qb)
                if pr == 0 and qb == 0:
                    push_k(1, range(NQB))
                elif pr == 0 and qb == 1:
                    push_k(2, [0, 1])
                elif pr == 0 and qb == 2:
                    push_k(2, [2, 3])
                elif pr == 0 and qb == 3:
                    push_h1(0)
                    push_gate(0, 0)
                    push_tail(0, 0)
                elif pr == 1:
                    if qb < 3:
                        push_h1(qb + 1)
                        push_gate(0, qb + 1)
                        push_tail(0, qb + 1)
                        push_k(3, [qb])
                    else:
                        push_gate(1, 0)
                        push_tail(1, 0)
                        push_k(3, [3])
                elif pr == 2:
                    if qb < 3:
                        push_gate(1, qb + 1)
                        push_tail(1, qb + 1)
                    else:
                        push_gate(2, 0)
                        push_tail(2, 0)
                elif pr == 3:
                    if qb < 3:
                        push_gate(2, qb + 1)
                        push_tail(2, qb + 1)
                        push_gate(3, qb)
                        push_tail(3, qb)
                        push_outproj(qb)
                    else:
                        push_gate(3, 3)
                        push_tail(3, 3)
                        push_outproj(3)
            drain()

    nc.compile()
    return nc


_CACHE = {}


def _get_nc():
    if "nc" not in _CACHE:
        _CACHE["nc"] = build_nc()
    return _CACHE["nc"]


def _bf16_t(a):
    """transpose + cast to contiguous bf16"""
    return np.ascontiguousarray(
        np.asarray(a, np.float32).T).astype(ml_dtypes.bfloat16)


def kernel(query, Wq, bq, Wk, bk, Wv, bv, Wo, bo,
           Wm1, bm1, Wm2, bm2,
           dopamine, serotonin, norepinephrine, acetylcholine,
           attn_scale, attn_bias):
    B, S, E = 4, 2048, 1024
    CH = 512
    nc = _get_nc()

    query = np.asarray(query, np.float32)
    f32 = lambda a: np.ascontiguousarray(np.asarray(a, np.float32))
    scal_row = np.array([float(np.asarray(dopamine).reshape(-1)[0]),
                         float(np.asarray(serotonin).reshape(-1)[0]),
                         float(np.asarray(norepinephrine).reshape(-1)[0]),
                         float(np.asarray(acetylcholine).reshape(-1)[0]),
                         float(np.asarray(attn_scale).reshape(-1)[0]),
                         float(np.asarray(attn_bias).reshape(-1)[0]),
                         0.0, 0.0], np.float32)
    scal = np.tile(scal_row[None, :], (128, 1))
    D_ = 64
    sel = np.zeros((2, 128), np.float32)
    sel[0, 0:D_] = 1.0
    sel[1, D_:2 * D_] = 1.0

    wm1T = _bf16_t(Wm1)
    Wo_np = np.asarray(Wo, np.float32)
    in_maps = []
    for core in range(8):
        b, g = core // 2, core % 2
        cg = slice(g * CH, (g + 1) * CH)
        in_maps.append({
            "xT": _bf16_t(query[b]),
            "wqT": _bf16_t(np.asarray(Wq, np.float32)[cg]),
            "wkT": _bf16_t(np.asarray(Wk, np.float32)[cg]),
            "wvT": _bf16_t(np.asarray(Wv, np.float32)[cg]),
            "wm1T": wm1T,
            "wm2T": _bf16_t(np.asarray(Wm2, np.float32)[cg]),
            "wo": _bf16_t(Wo_np[:, cg]),
            "bq": f32(np.asarray(bq, np.float32)[cg]),
            "bk": f32(np.asarray(bk, np.float32)[cg]),
            "bvr": np.ascontiguousarray(
                np.tile(np.asarray(bv, np.float32)[cg][None, :], (128, 1))),
            "bm1": f32(bm1),
            "bm2": f32(np.asarray(bm2, np.float32)[cg]),
            "scal": scal,
            "sel": sel,
        })

    res = run_bass_kernel_spmd(nc, in_maps, core_ids=list(range(8)))
    _CACHE["last_results"] = res

    bo_np = np.asarray(bo, np.float32)
    out = np.empty((B, S, E), np.float32)
    for b in range(B):
        out[b] = (res.results[2 * b]["out"].astype(np.float32)
                  + res.results[2 * b + 1]["out"].astype(np.float32) + bo_np)
    return out
